# revision 29
# baseline (speedup 1.0000x reference)
"""MultiHeadAttention with RoPE on 8 Trainium2 NeuronCores.

Sharding: batch (2) x head-group (4 heads each) -> 8 cores. Each core
computes q/k/v projections for its 4 heads of one batch element, causal
attention, and a partial output projection (row-shard of Wo). The host
sums the 4 partial outputs per batch element (the "all-reduce").

v2: all matmul operands in bf16 (fp32r runs the PE at half rate; the
harness tolerance is 2e-2), per-512-chunk RoPE for finer pipelining,
V projected channel-major like Q/K (16 LDWEIGHTS instead of 128) then
flipped token-major via DMA-transpose, per-kb score/exp/AV interleave
with both head-halves merged into single ACT exp instructions, fast
approximate reciprocal for softmax denominators, out-projection
interleaved per q-tile.

Device layout per core:
  - x.T (d-major, bf16) in SBUF; all projections contract over d.
  - Q/K produced channel-partition (Q.T layout); RoPE applied via DVE
    stream_shuffle (partition XOR-1) + cos/sin tables (bf16).
  - scores computed transposed (k on partitions, q on free) so the AV
    matmul needs no transposes.
  - softmax denominators come free from an extra ones-column in the
    V-stationary AV matmul (M=65); exp on ACT with causal suffix trim,
    triangle masking on GPSIMD affine_select.
"""

import numpy as np
import ml_dtypes

import concourse.bacc as bacc
import concourse.mybir as mybir
import concourse.tile as tile
from concourse.bass_utils import run_bass_kernel_spmd

F32 = mybir.dt.float32
BF16 = mybir.dt.bfloat16
EXP = mybir.ActivationFunctionType.Exp
LN = mybir.ActivationFunctionType.Ln

B, S, D = 2, 2048, 1024
H, HD = 16, 64
THETA = 10000.0
NCORES = 8
NH = 4          # heads per core
C = NH * HD     # 256 channels per core
P = 128
DC = D // P     # 8 contraction chunks
NQT = S // 512  # 4 q-tiles
NTB = S // P    # 16 token blocks

_NC_CACHE = None
LAST_RESULTS = None


def _build(phases=3):
    nc = bacc.Bacc(None)

    xT = nc.dram_tensor("xT", [D, S], BF16, kind="ExternalInput")
    wqT = nc.dram_tensor("wqT", [D, C], BF16, kind="ExternalInput")
    wkT = nc.dram_tensor("wkT", [D, C], BF16, kind="ExternalInput")
    wvT = nc.dram_tensor("wvT", [D, C], BF16, kind="ExternalInput")
    woT = nc.dram_tensor("woT", [C, D], BF16, kind="ExternalInput")
    cosP = nc.dram_tensor("cosP", [P, S], BF16, kind="ExternalInput")
    sinP = nc.dram_tensor("sinP", [P, S], BF16, kind="ExternalInput")
    out = nc.dram_tensor("out", [S, D], F32, kind="ExternalOutput")

    xT3 = xT.rearrange("(dc di) t -> di dc t", di=P)
    woT3 = woT.rearrange("(cp ci) o -> ci cp o", ci=P)

    XOR1 = [i ^ 1 for i in range(32)]

    with tile.TileContext(nc) as tc:
        with (
            tc.tile_pool(name="cn", bufs=1) as cn,        # constants
            tc.tile_pool(name="big", bufs=1) as big,      # long-lived tensors
        ):
            # ---- constants / big loads ----
            cos_sb = cn.tile([P, S], BF16, tag="cos")
            sin_sb = cn.tile([P, S], BF16, tag="sin")

            w_sb = {}
            for proj, wT in (("v", wvT), ("q", wqT), ("k", wkT)):
                wt = cn.tile([P, DC, C], BF16, tag=f"w{proj}")
                nc.sync.dma_start(wt[:], wT.rearrange("(dc di) c -> di dc c", di=P))
                w_sb[proj] = wt

            xt_sb = cn.tile([P, DC, S], BF16, tag="xt")
            for dc in range(DC):
                nc.sync.dma_start(xt_sb[:, dc, :], xT3[:, dc, :])

            nc.sync.dma_start(cos_sb[:], cosP[:])
            nc.sync.dma_start(sin_sb[:], sinP[:])
            wo_sb = cn.tile([P, 2, D], BF16, tag="wo")
            nc.sync.dma_start(wo_sb[:], woT3[:])

            ones_sb = cn.tile([P, NH], BF16, tag="ones")
            nc.gpsimd.memset(ones_sb[:], 1.0)

            qk_tiles = {}   # (proj, pair) -> (128, S) bf16 roped tile
            vp_tiles = []   # 16 x (128, NH, 65) bf16 tiles [V | ones]
            for tb in range(NTB):
                vp = big.tile([P, NH, 65], BF16, tag=f"vp{tb}")
                vp_tiles.append(vp)
                nc.scalar.copy(vp[:, :, 64:65], ones_sb[:, :, None])

            # ---- V projection first (streams behind the input DMA) ----
            # two waves of 8 token-blocks, dc-outer, so each matmul waits
            # only for its own xT chunk instead of the full load
            with tc.tile_pool(name="psV", bufs=8, space="PSUM") as psV:
                for wave in range(2):
                    tbs_w = range(8 * wave, 8 * wave + 8)
                    pss = {}
                    for tb in tbs_w:
                        pss[tb] = psV.tile([P, C], F32, tag="v",
                                           name=f"v{tb}")
                    for dc in range(DC):
                        for tb in tbs_w:
                            nc.tensor.matmul(
                                pss[tb][:], xt_sb[:, dc, tb * P:(tb + 1) * P],
                                w_sb["v"][:, dc, :],
                                start=(dc == 0), stop=(dc == DC - 1),
                                skip_group_check=True)
                    for tb in tbs_w:
                        nc.scalar.copy(
                            vp_tiles[tb][:, :, 0:HD],
                            pss[tb].rearrange("p (h c) -> p h c", c=HD))

            # ---- Q/K projections + rope ----
            with (
                tc.tile_pool(name="psQK", bufs=2, space="PSUM") as psQK,
                tc.tile_pool(name="shp", bufs=2) as shp,
            ):
                for proj, pair in (("q", 0), ("k", 0), ("q", 1), ("k", 1)):
                    dst = big.tile([P, S], BF16, tag=f"{proj}{pair}",
                                   name=f"{proj}{pair}")
                    qk_tiles[(proj, pair)] = dst
                    ps = psQK.tile([P, S], F32, tag="qk")
                    for dc in range(DC):
                        w = w_sb[proj][:, dc, pair * P:(pair + 1) * P]
                        for tt in range(NQT):
                            nc.tensor.matmul(
                                ps[:, tt * 512:(tt + 1) * 512], w,
                                xt_sb[:, dc, tt * 512:(tt + 1) * 512],
                                start=(dc == 0), stop=(dc == DC - 1))
                    for tt in range(NQT):
                        cs = slice(tt * 512, (tt + 1) * 512)
                        sh = shp.tile([P, 512], F32, tag="sh")
                        sh2 = shp.tile([P, 512], BF16, tag="sh2")
                        nc.vector.stream_shuffle(sh[:], ps[:, cs], XOR1)
                        nc.vector.tensor_mul(dst[:, cs], ps[:, cs], cos_sb[:, cs])
                        nc.gpsimd.tensor_mul(sh2[:], sh[:], sin_sb[:, cs])
                        nc.vector.tensor_add(dst[:, cs], dst[:, cs], sh2[:])

            if phases == 1:
                with tc.tile_pool(name="dbg", bufs=2) as dbg:
                    for i, t in enumerate(qk_tiles.values()):
                        d = dbg.tile([P, 1024], F32, tag="d")
                        nc.vector.tensor_copy(d[:], t[:, 0:1024])
                        nc.sync.dma_start(out[i * P:(i + 1) * P, 0:1024], d[:])
                    for tb in range(12):
                        d2 = dbg.tile([P, NH * 65], F32, tag="d2")
                        nc.vector.tensor_copy(
                            d2[:], vp_tiles[tb].rearrange("p h c -> p (h c)"))
                        nc.sync.dma_start(
                            out[512 + tb * P:512 + (tb + 1) * P, 0:NH * 65],
                            d2[:])

            # ---- attention + out-projection ----
            if phases >= 3:
              with (
                tc.tile_pool(name="psSC", bufs=2, space="PSUM") as psSC,  # 4 banks
                tc.tile_pool(name="psAV", bufs=2, space="PSUM") as psAV,  # 4 banks
                tc.tile_pool(name="ex", bufs=3) as ex,
                tc.tile_pool(name="nrm", bufs=2) as nrm,
                tc.tile_pool(name="ob", bufs=3) as ob,
            ):
                yt = {0: big.tile([P, S], BF16, tag="y0", name="y0"),
                      1: big.tile([P, S], BF16, tag="y1", name="y1")}

                def norm(pair, qt, av):
                    qs = slice(qt * 512, (qt + 1) * 512)
                    rec = nrm.tile([1, 2, 512], F32, tag="rec")
                    for o in range(2):
                        nc.vector.reciprocal(rec[0:1, o, :], av[64:65, o, :])
                        rb = nrm.tile([64, 512], F32, tag="rb")
                        nc.gpsimd.partition_broadcast(rb[:], rec[0:1, o, :])
                        nc.vector.tensor_mul(
                            yt[pair][64 * o:64 * o + 64, qs],
                            av[0:64, o, :], rb[:])

                def outproj_piece(tb, oc):
                    tbs = slice(tb * P, (tb + 1) * P)
                    pt = psSC.tile([P, 2, 512], F32, tag="sc",
                                   name=f"po{tb}_{oc}")
                    po = pt[:, 0, :]
                    for cp in range(2):
                        nc.tensor.matmul(
                            po[:], yt[cp][:, tbs],
                            wo_sb[:, cp, oc * 512:(oc + 1) * 512],
                            start=(cp == 0), stop=(cp == 1))
                    ot = ob.tile([P, 512], F32, tag="ot")
                    nc.vector.tensor_copy(ot[:], po[:])
                    nc.sync.dma_start(
                        out[tbs, oc * 512:(oc + 1) * 512], ot[:])

                # software-pipelined emission: score/exp of unit u+1 are
                # emitted before the AV matmuls of unit u so the in-order
                # PE queue always holds independent work while ACT runs
                # exp; outproj pieces trickle in 2-per-unit once queued.
                units = []
                for pair, qt in ([(0, qt) for qt in (3, 2, 1, 0)]
                                 + [(1, qt) for qt in (2, 1, 0, 3)]):
                    nkb = 4 * qt + 4
                    units += [(pair, qt, kb, nkb) for kb in range(nkb)]

                avs = {}
                opq = []       # deferred outproj pieces
                pend = None    # (pair, qt, kb, nkb, et, off, w_)

                def flush(p, drain=0):
                    pair, qt, kb, nkb, et, off, w_ = p
                    if kb == 0:
                        avs[(pair, qt)] = psAV.tile(
                            [P, 2, 512], F32, tag="av", name=f"av{pair}{qt}")
                    av = avs[(pair, qt)]
                    for o in range(2):
                        nc.tensor.matmul(
                            av[0:65, o, off:512],
                            vp_tiles[kb][:, 2 * pair + o, :],
                            et[:, o, 0:w_],
                            start=(kb == 0), stop=(kb == nkb - 1),
                            skip_group_check=True)
                    fresh = False
                    if kb == nkb - 1:
                        norm(pair, qt, av)
                        if pair == 1:
                            opq.extend((tb, oc)
                                       for tb in range(4 * qt, 4 * qt + 4)
                                       for oc in range(2))
                            fresh = True
                    if not fresh or drain:
                        for _ in range(drain if drain else 2):
                            if opq:
                                outproj_piece(*opq.pop(0))

                for pair, qt, kb, nkb in units:
                    off = max(0, (kb - 4 * qt) * P)
                    w_ = 512 - off
                    qtile = qk_tiles[("q", pair)]
                    ktile = qk_tiles[("k", pair)]
                    sc = psSC.tile([P, 2, 512], F32, tag="sc")
                    for o in range(2):
                        hs = slice(64 * o, 64 * o + 64)
                        nc.tensor.matmul(
                            sc[:, o, 0:w_],
                            ktile[hs, kb * P:(kb + 1) * P],
                            qtile[hs, qt * 512 + off:(qt + 1) * 512],
                            start=True, stop=True)
                    et = ex.tile([P, 2, 512], BF16, tag="e")
                    nc.scalar.activation(
                        et[:, :, 0:w_], sc[:, :, 0:w_], EXP, scale=0.125)
                    if kb >= 4 * qt:
                        for o in range(2):
                            nc.gpsimd.affine_select(
                                et[:, o, 0:P], et[:, o, 0:P],
                                [[1, P]], mybir.AluOpType.is_ge, 0.0,
                                base=0, channel_multiplier=-1)
                    if pend is not None:
                        flush(pend)
                    pend = (pair, qt, kb, nkb, et, off, w_)
                flush(pend, drain=len(opq) + 8)

    nc.finalize()
    return nc


def _prep_core_inputs(x, pos, Wq, Wk, Wv, Wo):
    """Per-core input dicts (host-side sharding + layout prep)."""
    inv_freq = THETA ** (-np.arange(0, HD, 2, dtype=np.float32) / HD)
    ang = pos.astype(np.float32)[:, None] * inv_freq[None, :]   # (S, 32)
    cos = np.cos(ang).astype(np.float32)                        # (S, 32)
    sin = np.sin(ang).astype(np.float32)
    p = np.arange(P)
    pairidx = (p % HD) // 2
    cosP = np.ascontiguousarray(cos[:, pairidx].T)              # (128, S)
    sgn = np.where(p % 2 == 0, -1.0, 1.0).astype(np.float32)
    sinP = np.ascontiguousarray(sin[:, pairidx].T * sgn[:, None])

    bf = ml_dtypes.bfloat16
    cosPb = cosP.astype(bf)
    sinPb = sinP.astype(bf)
    xTs = [np.ascontiguousarray(x[b].T).astype(bf) for b in range(B)]  # (D, S)
    maps = []
    for c in range(NCORES):
        b, g = divmod(c, NH)
        cs = slice(C * g, C * (g + 1))
        maps.append({
            "xT": xTs[b],
            "wqT": np.ascontiguousarray(Wq[cs, :].T).astype(bf),
            "wkT": np.ascontiguousarray(Wk[cs, :].T).astype(bf),
            "wvT": np.ascontiguousarray(Wv[cs, :].T).astype(bf),
            "woT": np.ascontiguousarray(Wo[:, cs].T).astype(bf),
            "cosP": cosPb,
            "sinP": sinPb,
        })
    return maps


def kernel(in_features, token_positions, Wq, Wk, Wv, Wo):
    global _NC_CACHE, LAST_RESULTS
    x = np.asarray(in_features, dtype=np.float32)
    pos = np.asarray(token_positions)
    Wq = np.asarray(Wq, dtype=np.float32)
    Wk = np.asarray(Wk, dtype=np.float32)
    Wv = np.asarray(Wv, dtype=np.float32)
    Wo = np.asarray(Wo, dtype=np.float32)

    if _NC_CACHE is None:
        _NC_CACHE = _build()
    maps = _prep_core_inputs(x, pos, Wq, Wk, Wv, Wo)
    res = run_bass_kernel_spmd(_NC_CACHE, maps, core_ids=list(range(NCORES)))
    LAST_RESULTS = res
    parts = [r["out"] for r in res.results]
    outb = [parts[4 * b] + parts[4 * b + 1] + parts[4 * b + 2] + parts[4 * b + 3]
            for b in range(B)]
    return np.stack(outb).astype(np.float32)


if __name__ == "__main__":
    rng = np.random.default_rng(0)
    x = rng.standard_normal((B, S, D), dtype=np.float32)
    o = kernel(x, np.arange(S, dtype=np.int32),
               *(rng.standard_normal((D, D), dtype=np.float32) / 32
                 for _ in range(4)))
    print(o.shape, o.dtype)


# revision 31
# speedup vs baseline: 1.0449x; 1.0449x over previous
"""MultiHeadAttention with RoPE on 8 Trainium2 NeuronCores.

Sharding: batch (2) x head-group (4 heads each) -> 8 cores. Each core
computes q/k/v projections for its 4 heads of one batch element, causal
attention, and a partial output projection (row-shard of Wo). The host
sums the 4 partial outputs per batch element (the "all-reduce").

v2: all matmul operands in bf16 (fp32r runs the PE at half rate; the
harness tolerance is 2e-2), per-512-chunk RoPE for finer pipelining,
V projected channel-major like Q/K (16 LDWEIGHTS instead of 128) then
flipped token-major via DMA-transpose, per-kb score/exp/AV interleave
with both head-halves merged into single ACT exp instructions, fast
approximate reciprocal for softmax denominators, out-projection
interleaved per q-tile.

Device layout per core:
  - x.T (d-major, bf16) in SBUF; all projections contract over d.
  - Q/K produced channel-partition (Q.T layout); RoPE applied via DVE
    stream_shuffle (partition XOR-1) + cos/sin tables (bf16).
  - scores computed transposed (k on partitions, q on free) so the AV
    matmul needs no transposes.
  - softmax denominators come free from an extra ones-column in the
    V-stationary AV matmul (M=65); exp on ACT with causal suffix trim,
    triangle masking on GPSIMD affine_select.
"""

import numpy as np
import ml_dtypes

import concourse.bacc as bacc
import concourse.mybir as mybir
import concourse.tile as tile
from concourse.bass_utils import run_bass_kernel_spmd

F32 = mybir.dt.float32
BF16 = mybir.dt.bfloat16
EXP = mybir.ActivationFunctionType.Exp
LN = mybir.ActivationFunctionType.Ln

B, S, D = 2, 2048, 1024
H, HD = 16, 64
THETA = 10000.0
NCORES = 8
NH = 4          # heads per core
C = NH * HD     # 256 channels per core
P = 128
DC = D // P     # 8 contraction chunks
NQT = S // 512  # 4 q-tiles
NTB = S // P    # 16 token blocks

_NC_CACHE = None
LAST_RESULTS = None


def _build(phases=3):
    nc = bacc.Bacc(None)

    xT = nc.dram_tensor("xT", [D, S], BF16, kind="ExternalInput")
    wqT = nc.dram_tensor("wqT", [D, C], BF16, kind="ExternalInput")
    wkT = nc.dram_tensor("wkT", [D, C], BF16, kind="ExternalInput")
    wvT = nc.dram_tensor("wvT", [D, C], BF16, kind="ExternalInput")
    woT = nc.dram_tensor("woT", [C, D], BF16, kind="ExternalInput")
    cosP = nc.dram_tensor("cosP", [P, S], BF16, kind="ExternalInput")
    sinP = nc.dram_tensor("sinP", [P, S], BF16, kind="ExternalInput")
    out = nc.dram_tensor("out", [S, D], F32, kind="ExternalOutput")

    xT3 = xT.rearrange("(dc di) t -> di dc t", di=P)
    woT3 = woT.rearrange("(cp ci) o -> ci cp o", ci=P)

    XOR1 = [i ^ 1 for i in range(32)]

    with tile.TileContext(nc) as tc:
        with (
            tc.tile_pool(name="cn", bufs=1) as cn,        # constants
            tc.tile_pool(name="big", bufs=1) as big,      # long-lived tensors
        ):
            # ---- constants / big loads ----
            cos_sb = cn.tile([P, S], BF16, tag="cos")
            sin_sb = cn.tile([P, S], BF16, tag="sin")

            w_sb = {}
            for proj, wT in (("v", wvT), ("q", wqT), ("k", wkT)):
                wt = cn.tile([P, DC, C], BF16, tag=f"w{proj}")
                nc.sync.dma_start(wt[:], wT.rearrange("(dc di) c -> di dc c", di=P))
                w_sb[proj] = wt

            xt_sb = cn.tile([P, DC, S], BF16, tag="xt")
            for dc in range(DC):
                nc.sync.dma_start(xt_sb[:, dc, :], xT3[:, dc, :])

            nc.sync.dma_start(cos_sb[:], cosP[:])
            nc.sync.dma_start(sin_sb[:], sinP[:])
            wo_sb = cn.tile([P, 2, D], BF16, tag="wo")
            nc.sync.dma_start(wo_sb[:], woT3[:])

            ones_sb = cn.tile([P, NH], BF16, tag="ones")
            nc.gpsimd.memset(ones_sb[:], 1.0)

            qk_tiles = {}   # (proj, pair) -> (128, S) bf16 roped tile
            vp_tiles = []   # 16 x (128, NH, 65) bf16 tiles [V | ones]
            for tb in range(NTB):
                vp = big.tile([P, NH, 65], BF16, tag=f"vp{tb}")
                vp_tiles.append(vp)
                nc.scalar.copy(vp[:, :, 64:65], ones_sb[:, :, None])

            # ---- V projection first (streams behind the input DMA) ----
            with tc.tile_pool(name="psV", bufs=3, space="PSUM") as psV:
                for tb in range(NTB):
                    ps = psV.tile([P, C], F32, tag="v")
                    for dc in range(DC):
                        nc.tensor.matmul(
                            ps[:], xt_sb[:, dc, tb * P:(tb + 1) * P],
                            w_sb["v"][:, dc, :],
                            start=(dc == 0), stop=(dc == DC - 1))
                    nc.vector.tensor_copy(
                        vp_tiles[tb][:, :, 0:HD],
                        ps.rearrange("p (h c) -> p h c", c=HD))

            # ---- Q/K projections + rope ----
            with (
                tc.tile_pool(name="psQK", bufs=2, space="PSUM") as psQK,
                tc.tile_pool(name="shp", bufs=2) as shp,
            ):
                for proj, pair in (("q", 0), ("k", 0), ("q", 1), ("k", 1)):
                    dst = big.tile([P, S], BF16, tag=f"{proj}{pair}",
                                   name=f"{proj}{pair}")
                    qk_tiles[(proj, pair)] = dst
                    ps = psQK.tile([P, S], F32, tag="qk")
                    for dc in range(DC):
                        w = w_sb[proj][:, dc, pair * P:(pair + 1) * P]
                        for tt in range(NQT):
                            nc.tensor.matmul(
                                ps[:, tt * 512:(tt + 1) * 512], w,
                                xt_sb[:, dc, tt * 512:(tt + 1) * 512],
                                start=(dc == 0), stop=(dc == DC - 1))
                    for tt in range(NQT):
                        cs = slice(tt * 512, (tt + 1) * 512)
                        sh = shp.tile([P, 512], F32, tag="sh")
                        sh2 = shp.tile([P, 512], BF16, tag="sh2")
                        nc.vector.stream_shuffle(sh[:], ps[:, cs], XOR1)
                        nc.vector.tensor_mul(dst[:, cs], ps[:, cs], cos_sb[:, cs])
                        nc.gpsimd.tensor_mul(sh2[:], sh[:], sin_sb[:, cs])
                        nc.vector.tensor_add(dst[:, cs], dst[:, cs], sh2[:])

            if phases == 1:
                with tc.tile_pool(name="dbg", bufs=2) as dbg:
                    for i, t in enumerate(qk_tiles.values()):
                        d = dbg.tile([P, 1024], F32, tag="d")
                        nc.vector.tensor_copy(d[:], t[:, 0:1024])
                        nc.sync.dma_start(out[i * P:(i + 1) * P, 0:1024], d[:])
                    for tb in range(12):
                        d2 = dbg.tile([P, NH * 65], F32, tag="d2")
                        nc.vector.tensor_copy(
                            d2[:], vp_tiles[tb].rearrange("p h c -> p (h c)"))
                        nc.sync.dma_start(
                            out[512 + tb * P:512 + (tb + 1) * P, 0:NH * 65],
                            d2[:])

            # ---- attention + out-projection ----
            if phases >= 3:
              with (
                tc.tile_pool(name="psSC", bufs=2, space="PSUM") as psSC,  # 4 banks
                tc.tile_pool(name="psAV", bufs=2, space="PSUM") as psAV,  # 4 banks
                tc.tile_pool(name="ex", bufs=3) as ex,
                tc.tile_pool(name="nrm", bufs=2) as nrm,
                tc.tile_pool(name="ob", bufs=3) as ob,
            ):
                yt = {0: big.tile([P, S], BF16, tag="y0", name="y0"),
                      1: big.tile([P, S], BF16, tag="y1", name="y1")}

                def norm(pair, qt, av):
                    qs = slice(qt * 512, (qt + 1) * 512)
                    rec = nrm.tile([1, 2, 512], F32, tag="rec")
                    for o in range(2):
                        nc.vector.reciprocal(rec[0:1, o, :], av[64:65, o, :])
                        rb = nrm.tile([64, 512], F32, tag="rb")
                        nc.gpsimd.partition_broadcast(rb[:], rec[0:1, o, :])
                        nc.vector.tensor_mul(
                            yt[pair][64 * o:64 * o + 64, qs],
                            av[0:64, o, :], rb[:])

                def outproj_piece(tb, oc):
                    tbs = slice(tb * P, (tb + 1) * P)
                    pt = psSC.tile([P, 2, 512], F32, tag="sc",
                                   name=f"po{tb}_{oc}")
                    po = pt[:, 0, :]
                    for cp in range(2):
                        nc.tensor.matmul(
                            po[:], yt[cp][:, tbs],
                            wo_sb[:, cp, oc * 512:(oc + 1) * 512],
                            start=(cp == 0), stop=(cp == 1))
                    ot = ob.tile([P, 512], F32, tag="ot")
                    nc.vector.tensor_copy(ot[:], po[:])
                    nc.sync.dma_start(
                        out[tbs, oc * 512:(oc + 1) * 512], ot[:])

                # software-pipelined emission: score/exp of unit u+1 are
                # emitted before the AV matmuls of unit u so the in-order
                # PE queue always holds independent work while ACT runs
                # exp; outproj pieces trickle in 2-per-unit once queued.
                units = []
                for pair, qt in ([(0, qt) for qt in range(NQT)]
                                 + [(1, qt) for qt in reversed(range(NQT))]):
                    nkb = 4 * qt + 4
                    units += [(pair, qt, kb, nkb) for kb in range(nkb)]

                avs = {}
                opq = []       # deferred outproj pieces
                pend = None    # (pair, qt, kb, nkb, et, off, w_)

                def flush(p, drain=0):
                    pair, qt, kb, nkb, et, off, w_ = p
                    if kb == 0:
                        avs[(pair, qt)] = psAV.tile(
                            [P, 2, 512], F32, tag="av", name=f"av{pair}{qt}")
                    av = avs[(pair, qt)]
                    for o in range(2):
                        nc.tensor.matmul(
                            av[0:65, o, off:512],
                            vp_tiles[kb][:, 2 * pair + o, :],
                            et[:, o, 0:w_],
                            start=(kb == 0), stop=(kb == nkb - 1),
                            skip_group_check=True)
                    fresh = False
                    if kb == nkb - 1:
                        norm(pair, qt, av)
                        if pair == 1:
                            opq.extend((tb, oc)
                                       for tb in range(4 * qt, 4 * qt + 4)
                                       for oc in range(2))
                            fresh = True
                    if not fresh or drain:
                        for _ in range(drain if drain else 2):
                            if opq:
                                outproj_piece(*opq.pop(0))

                for pair, qt, kb, nkb in units:
                    off = max(0, (kb - 4 * qt) * P)
                    w_ = 512 - off
                    qtile = qk_tiles[("q", pair)]
                    ktile = qk_tiles[("k", pair)]
                    sc = psSC.tile([P, 2, 512], F32, tag="sc")
                    for o in range(2):
                        hs = slice(64 * o, 64 * o + 64)
                        nc.tensor.matmul(
                            sc[:, o, 0:w_],
                            ktile[hs, kb * P:(kb + 1) * P],
                            qtile[hs, qt * 512 + off:(qt + 1) * 512],
                            start=True, stop=True)
                    et = ex.tile([P, 2, 512], BF16, tag="e")
                    nc.scalar.activation(
                        et[:, :, 0:w_], sc[:, :, 0:w_], EXP, scale=0.125)
                    if kb >= 4 * qt:
                        for o in range(2):
                            nc.gpsimd.affine_select(
                                et[:, o, 0:P], et[:, o, 0:P],
                                [[1, P]], mybir.AluOpType.is_ge, 0.0,
                                base=0, channel_multiplier=-1)
                    if pend is not None:
                        flush(pend)
                    pend = (pair, qt, kb, nkb, et, off, w_)
                flush(pend, drain=len(opq) + 8)

    nc.finalize()
    return nc


def _prep_core_inputs(x, pos, Wq, Wk, Wv, Wo):
    """Per-core input dicts (host-side sharding + layout prep)."""
    inv_freq = THETA ** (-np.arange(0, HD, 2, dtype=np.float32) / HD)
    ang = pos.astype(np.float32)[:, None] * inv_freq[None, :]   # (S, 32)
    cos = np.cos(ang).astype(np.float32)                        # (S, 32)
    sin = np.sin(ang).astype(np.float32)
    p = np.arange(P)
    pairidx = (p % HD) // 2
    cosP = np.ascontiguousarray(cos[:, pairidx].T)              # (128, S)
    sgn = np.where(p % 2 == 0, -1.0, 1.0).astype(np.float32)
    sinP = np.ascontiguousarray(sin[:, pairidx].T * sgn[:, None])

    bf = ml_dtypes.bfloat16
    cosPb = cosP.astype(bf)
    sinPb = sinP.astype(bf)
    xTs = [np.ascontiguousarray(x[b].T).astype(bf) for b in range(B)]  # (D, S)
    maps = []
    for c in range(NCORES):
        b, g = divmod(c, NH)
        cs = slice(C * g, C * (g + 1))
        maps.append({
            "xT": xTs[b],
            "wqT": np.ascontiguousarray(Wq[cs, :].T).astype(bf),
            "wkT": np.ascontiguousarray(Wk[cs, :].T).astype(bf),
            "wvT": np.ascontiguousarray(Wv[cs, :].T).astype(bf),
            "woT": np.ascontiguousarray(Wo[:, cs].T).astype(bf),
            "cosP": cosPb,
            "sinP": sinPb,
        })
    return maps


def kernel(in_features, token_positions, Wq, Wk, Wv, Wo):
    global _NC_CACHE, LAST_RESULTS
    x = np.asarray(in_features, dtype=np.float32)
    pos = np.asarray(token_positions)
    Wq = np.asarray(Wq, dtype=np.float32)
    Wk = np.asarray(Wk, dtype=np.float32)
    Wv = np.asarray(Wv, dtype=np.float32)
    Wo = np.asarray(Wo, dtype=np.float32)

    if _NC_CACHE is None:
        _NC_CACHE = _build()
    maps = _prep_core_inputs(x, pos, Wq, Wk, Wv, Wo)
    res = run_bass_kernel_spmd(_NC_CACHE, maps, core_ids=list(range(NCORES)))
    LAST_RESULTS = res
    parts = [r["out"] for r in res.results]
    outb = [parts[4 * b] + parts[4 * b + 1] + parts[4 * b + 2] + parts[4 * b + 3]
            for b in range(B)]
    return np.stack(outb).astype(np.float32)


if __name__ == "__main__":
    rng = np.random.default_rng(0)
    x = rng.standard_normal((B, S, D), dtype=np.float32)
    o = kernel(x, np.arange(S, dtype=np.int32),
               *(rng.standard_normal((D, D), dtype=np.float32) / 32
                 for _ in range(4)))
    print(o.shape, o.dtype)


# revision 34
# speedup vs baseline: 1.0459x; 1.0010x over previous
"""MultiHeadAttention with RoPE on 8 Trainium2 NeuronCores.

Sharding: batch (2) x head-group (4 heads each) -> 8 cores. Each core
computes q/k/v projections for its 4 heads of one batch element, causal
attention, and a partial output projection (row-shard of Wo). The host
sums the 4 partial outputs per batch element (the "all-reduce").

v2: all matmul operands in bf16 (fp32r runs the PE at half rate; the
harness tolerance is 2e-2), per-512-chunk RoPE for finer pipelining,
V projected channel-major like Q/K (16 LDWEIGHTS instead of 128) then
flipped token-major via DMA-transpose, per-kb score/exp/AV interleave
with both head-halves merged into single ACT exp instructions, fast
approximate reciprocal for softmax denominators, out-projection
interleaved per q-tile.

Device layout per core:
  - x.T (d-major, bf16) in SBUF; all projections contract over d.
  - Q/K produced channel-partition (Q.T layout); RoPE applied via DVE
    stream_shuffle (partition XOR-1) + cos/sin tables (bf16).
  - scores computed transposed (k on partitions, q on free) so the AV
    matmul needs no transposes.
  - softmax denominators come free from an extra ones-column in the
    V-stationary AV matmul (M=65); exp on ACT with causal suffix trim,
    triangle masking on GPSIMD affine_select.
"""

import numpy as np
import ml_dtypes

import concourse.bacc as bacc
import concourse.mybir as mybir
import concourse.tile as tile
from concourse.bass_utils import run_bass_kernel_spmd

F32 = mybir.dt.float32
BF16 = mybir.dt.bfloat16
EXP = mybir.ActivationFunctionType.Exp
LN = mybir.ActivationFunctionType.Ln

B, S, D = 2, 2048, 1024
H, HD = 16, 64
THETA = 10000.0
NCORES = 8
NH = 4          # heads per core
C = NH * HD     # 256 channels per core
P = 128
DC = D // P     # 8 contraction chunks
NQT = S // 512  # 4 q-tiles
NTB = S // P    # 16 token blocks

_NC_CACHE = None
LAST_RESULTS = None


def _build(phases=3):
    nc = bacc.Bacc(None)

    xT = nc.dram_tensor("xT", [D, S], BF16, kind="ExternalInput")
    wqT = nc.dram_tensor("wqT", [D, C], BF16, kind="ExternalInput")
    wkT = nc.dram_tensor("wkT", [D, C], BF16, kind="ExternalInput")
    wvT = nc.dram_tensor("wvT", [D, C], BF16, kind="ExternalInput")
    woT = nc.dram_tensor("woT", [C, D], BF16, kind="ExternalInput")
    cosP = nc.dram_tensor("cosP", [P, S], BF16, kind="ExternalInput")
    sinP = nc.dram_tensor("sinP", [P, S], BF16, kind="ExternalInput")
    out = nc.dram_tensor("out", [S, D], F32, kind="ExternalOutput")

    xT3 = xT.rearrange("(dc di) t -> di dc t", di=P)
    woT3 = woT.rearrange("(cp ci) o -> ci cp o", ci=P)

    XOR1 = [i ^ 1 for i in range(32)]

    with tile.TileContext(nc) as tc:
        with (
            tc.tile_pool(name="cn", bufs=1) as cn,        # constants
            tc.tile_pool(name="big", bufs=1) as big,      # long-lived tensors
        ):
            # ---- constants / big loads ----
            cos_sb = cn.tile([P, S], BF16, tag="cos")
            sin_sb = cn.tile([P, S], BF16, tag="sin")

            w_sb = {}
            for proj, wT in (("v", wvT), ("q", wqT), ("k", wkT)):
                wt = cn.tile([P, DC, C], BF16, tag=f"w{proj}")
                nc.sync.dma_start(wt[:], wT.rearrange("(dc di) c -> di dc c", di=P))
                w_sb[proj] = wt

            xt_sb = cn.tile([P, DC, S], BF16, tag="xt")
            for dc in range(DC):
                nc.sync.dma_start(xt_sb[:, dc, :], xT3[:, dc, :])

            nc.sync.dma_start(cos_sb[:], cosP[:])
            nc.sync.dma_start(sin_sb[:], sinP[:])
            wo_sb = cn.tile([P, 2, D], BF16, tag="wo")
            nc.sync.dma_start(wo_sb[:], woT3[:])

            ones_sb = cn.tile([P, NH], BF16, tag="ones")
            nc.gpsimd.memset(ones_sb[:], 1.0)

            qk_tiles = {}   # (proj, pair) -> (128, S) bf16 roped tile
            vp_tiles = []   # 16 x (128, NH, 65) bf16 tiles [V | ones]
            for tb in range(NTB):
                vp = big.tile([P, NH, 65], BF16, tag=f"vp{tb}")
                vp_tiles.append(vp)
                nc.scalar.copy(vp[:, :, 64:65], ones_sb[:, :, None])

            # ---- V projection first (streams behind the input DMA) ----
            with tc.tile_pool(name="psV", bufs=3, space="PSUM") as psV:
                for tb in range(NTB):
                    ps = psV.tile([P, C], F32, tag="v")
                    for dc in range(DC):
                        nc.tensor.matmul(
                            ps[:], xt_sb[:, dc, tb * P:(tb + 1) * P],
                            w_sb["v"][:, dc, :],
                            start=(dc == 0), stop=(dc == DC - 1))
                    nc.vector.tensor_copy(
                        vp_tiles[tb][:, :, 0:HD],
                        ps.rearrange("p (h c) -> p h c", c=HD))

            # ---- Q/K projections + rope ----
            with (
                tc.tile_pool(name="psQK", bufs=2, space="PSUM") as psQK,
                tc.tile_pool(name="shp", bufs=2) as shp,
            ):
                for proj, pair in (("q", 0), ("k", 0), ("q", 1), ("k", 1)):
                    dst = big.tile([P, S], BF16, tag=f"{proj}{pair}",
                                   name=f"{proj}{pair}")
                    qk_tiles[(proj, pair)] = dst
                    ps = psQK.tile([P, S], F32, tag="qk")
                    for dc in range(DC):
                        w = w_sb[proj][:, dc, pair * P:(pair + 1) * P]
                        for tt in range(NQT):
                            nc.tensor.matmul(
                                ps[:, tt * 512:(tt + 1) * 512], w,
                                xt_sb[:, dc, tt * 512:(tt + 1) * 512],
                                start=(dc == 0), stop=(dc == DC - 1))
                    for tt in range(NQT):
                        cs = slice(tt * 512, (tt + 1) * 512)
                        sh = shp.tile([P, 512], F32, tag="sh")
                        sh2 = shp.tile([P, 512], BF16, tag="sh2")
                        nc.vector.stream_shuffle(sh[:], ps[:, cs], XOR1)
                        nc.vector.tensor_mul(dst[:, cs], ps[:, cs], cos_sb[:, cs])
                        nc.gpsimd.tensor_mul(sh2[:], sh[:], sin_sb[:, cs])
                        nc.vector.tensor_add(dst[:, cs], dst[:, cs], sh2[:])

            if phases == 1:
                with tc.tile_pool(name="dbg", bufs=2) as dbg:
                    for i, t in enumerate(qk_tiles.values()):
                        d = dbg.tile([P, 1024], F32, tag="d")
                        nc.vector.tensor_copy(d[:], t[:, 0:1024])
                        nc.sync.dma_start(out[i * P:(i + 1) * P, 0:1024], d[:])
                    for tb in range(12):
                        d2 = dbg.tile([P, NH * 65], F32, tag="d2")
                        nc.vector.tensor_copy(
                            d2[:], vp_tiles[tb].rearrange("p h c -> p (h c)"))
                        nc.sync.dma_start(
                            out[512 + tb * P:512 + (tb + 1) * P, 0:NH * 65],
                            d2[:])

            # ---- attention + out-projection ----
            if phases >= 3:
              with (
                tc.tile_pool(name="psSC", bufs=2, space="PSUM") as psSC,  # 4 banks
                tc.tile_pool(name="psAV", bufs=2, space="PSUM") as psAV,  # 4 banks
                tc.tile_pool(name="ex", bufs=3) as ex,
                tc.tile_pool(name="nrm", bufs=2) as nrm,
                tc.tile_pool(name="ob", bufs=3) as ob,
            ):
                yt = {0: big.tile([P, S], BF16, tag="y0", name="y0"),
                      1: big.tile([P, S], BF16, tag="y1", name="y1")}

                def norm(pair, qt, av):
                    qs = slice(qt * 512, (qt + 1) * 512)
                    rec = nrm.tile([1, 2, 512], F32, tag="rec")
                    for o in range(2):
                        nc.vector.reciprocal(rec[0:1, o, :], av[64:65, o, :])
                        rb = nrm.tile([64, 512], F32, tag="rb")
                        nc.gpsimd.partition_broadcast(rb[:], rec[0:1, o, :])
                        nc.vector.tensor_mul(
                            yt[pair][64 * o:64 * o + 64, qs],
                            av[0:64, o, :], rb[:])

                def outproj_piece(tb):
                    # whole token-block: 4 matmuls, one eviction, one DMA
                    tbs = slice(tb * P, (tb + 1) * P)
                    pt = psSC.tile([P, 2, 512], F32, tag="sc",
                                   name=f"po{tb}")
                    for oc in range(2):
                        for cp in range(2):
                            nc.tensor.matmul(
                                pt[:, oc, :], yt[cp][:, tbs],
                                wo_sb[:, cp, oc * 512:(oc + 1) * 512],
                                start=(cp == 0), stop=(cp == 1))
                    ot = ob.tile([P, 1024], F32, tag="ot")
                    nc.vector.tensor_copy(
                        ot[:], pt.rearrange("p a b -> p (a b)"))
                    nc.sync.dma_start(out[tbs, :], ot[:])

                # software-pipelined emission: score/exp of unit u+1 are
                # emitted before the AV matmuls of unit u so the in-order
                # PE queue always holds independent work while ACT runs
                # exp; outproj pieces trickle in 2-per-unit once queued.
                units = []
                for pair, qt in ([(0, qt) for qt in range(NQT)]
                                 + [(1, qt) for qt in reversed(range(NQT))]):
                    nkb = 4 * qt + 4
                    units += [(pair, qt, kb, nkb) for kb in range(nkb)]

                avs = {}
                opq = []       # deferred outproj pieces
                pend = None    # (pair, qt, kb, nkb, et, off, w_)

                def flush(p, drain=0):
                    pair, qt, kb, nkb, et, off, w_ = p
                    if kb == 0:
                        avs[(pair, qt)] = psAV.tile(
                            [P, 2, 512], F32, tag="av", name=f"av{pair}{qt}")
                    av = avs[(pair, qt)]
                    for o in range(2):
                        nc.tensor.matmul(
                            av[0:65, o, off:512],
                            vp_tiles[kb][:, 2 * pair + o, :],
                            et[:, o, 0:w_],
                            start=(kb == 0), stop=(kb == nkb - 1),
                            skip_group_check=True)
                    fresh = False
                    if kb == nkb - 1:
                        norm(pair, qt, av)
                        if pair == 1:
                            opq.extend(range(4 * qt, 4 * qt + 4))
                            fresh = True
                    if not fresh or drain:
                        for _ in range(drain if drain else 1):
                            if opq:
                                outproj_piece(opq.pop(0))

                for pair, qt, kb, nkb in units:
                    off = max(0, (kb - 4 * qt) * P)
                    w_ = 512 - off
                    qtile = qk_tiles[("q", pair)]
                    ktile = qk_tiles[("k", pair)]
                    sc = psSC.tile([P, 2, 512], F32, tag="sc")
                    for o in range(2):
                        hs = slice(64 * o, 64 * o + 64)
                        nc.tensor.matmul(
                            sc[:, o, 0:w_],
                            ktile[hs, kb * P:(kb + 1) * P],
                            qtile[hs, qt * 512 + off:(qt + 1) * 512],
                            start=True, stop=True)
                    et = ex.tile([P, 2, 512], BF16, tag="e")
                    nc.scalar.activation(
                        et[:, :, 0:w_], sc[:, :, 0:w_], EXP, scale=0.125)
                    if kb >= 4 * qt:
                        for o in range(2):
                            nc.gpsimd.affine_select(
                                et[:, o, 0:P], et[:, o, 0:P],
                                [[1, P]], mybir.AluOpType.is_ge, 0.0,
                                base=0, channel_multiplier=-1)
                    if pend is not None:
                        flush(pend)
                    pend = (pair, qt, kb, nkb, et, off, w_)
                flush(pend, drain=len(opq) + 4)

    nc.finalize()
    return nc


def _prep_core_inputs(x, pos, Wq, Wk, Wv, Wo):
    """Per-core input dicts (host-side sharding + layout prep)."""
    inv_freq = THETA ** (-np.arange(0, HD, 2, dtype=np.float32) / HD)
    ang = pos.astype(np.float32)[:, None] * inv_freq[None, :]   # (S, 32)
    cos = np.cos(ang).astype(np.float32)                        # (S, 32)
    sin = np.sin(ang).astype(np.float32)
    p = np.arange(P)
    pairidx = (p % HD) // 2
    cosP = np.ascontiguousarray(cos[:, pairidx].T)              # (128, S)
    sgn = np.where(p % 2 == 0, -1.0, 1.0).astype(np.float32)
    sinP = np.ascontiguousarray(sin[:, pairidx].T * sgn[:, None])

    bf = ml_dtypes.bfloat16
    cosPb = cosP.astype(bf)
    sinPb = sinP.astype(bf)
    xTs = [np.ascontiguousarray(x[b].T).astype(bf) for b in range(B)]  # (D, S)
    maps = []
    for c in range(NCORES):
        b, g = divmod(c, NH)
        cs = slice(C * g, C * (g + 1))
        maps.append({
            "xT": xTs[b],
            "wqT": np.ascontiguousarray(Wq[cs, :].T).astype(bf),
            "wkT": np.ascontiguousarray(Wk[cs, :].T).astype(bf),
            "wvT": np.ascontiguousarray(Wv[cs, :].T).astype(bf),
            "woT": np.ascontiguousarray(Wo[:, cs].T).astype(bf),
            "cosP": cosPb,
            "sinP": sinPb,
        })
    return maps


def kernel(in_features, token_positions, Wq, Wk, Wv, Wo):
    global _NC_CACHE, LAST_RESULTS
    x = np.asarray(in_features, dtype=np.float32)
    pos = np.asarray(token_positions)
    Wq = np.asarray(Wq, dtype=np.float32)
    Wk = np.asarray(Wk, dtype=np.float32)
    Wv = np.asarray(Wv, dtype=np.float32)
    Wo = np.asarray(Wo, dtype=np.float32)

    if _NC_CACHE is None:
        _NC_CACHE = _build()
    maps = _prep_core_inputs(x, pos, Wq, Wk, Wv, Wo)
    res = run_bass_kernel_spmd(_NC_CACHE, maps, core_ids=list(range(NCORES)))
    LAST_RESULTS = res
    parts = [r["out"] for r in res.results]
    outb = [parts[4 * b] + parts[4 * b + 1] + parts[4 * b + 2] + parts[4 * b + 3]
            for b in range(B)]
    return np.stack(outb).astype(np.float32)


if __name__ == "__main__":
    rng = np.random.default_rng(0)
    x = rng.standard_normal((B, S, D), dtype=np.float32)
    o = kernel(x, np.arange(S, dtype=np.int32),
               *(rng.standard_normal((D, D), dtype=np.float32) / 32
                 for _ in range(4)))
    print(o.shape, o.dtype)
